# revision 14
# baseline (speedup 1.0000x reference)
"""Trainium2 Bass kernel for nn_BottleneckFFN.

Computes y = LayerNorm(GELU(x @ W1.T + b1) @ W2.T + b2) * gamma + beta
for x of shape (128, 2048, 256), W1 (8, 256), W2 (8, 8), LN over the
trailing 8 channels.  Pure data parallel over 8 NeuronCores: the
128*2048 = 262144 token rows are split into 8 shards of 32768 tokens;
the tiny weights are replicated.

Token map inside a shard: t = p*256 + r*32 + f (partition p owns 256
consecutive tokens; round r covers 32 of them per partition).

Per-core dataflow (8 rounds of 4096 tokens):
  1. One SWDGE (gpsimd) DMA per round casts 4 MB of fp32 x rows to
     bf16 on the fly (32 KB contiguous per-partition descriptors --
     the most efficient size measured on this part).  The HBM read is
     the binding resource (~90-100 us/core at measured engine rates).
     Round 0 instead runs through the Sync HWDGE queue as fp32 in 4
     quarters + DVE casts (the Pool engine boots ~5 us later than
     Sync; this starts the stream at ~2.6 us).  Round 7 loads in 4
     SWDGE quarters so the drain pipeline gets its data piecewise.
  2. One DVE 32x32 block transpose per round over the bf16 tile
     VIEWED AS uint32 (adjacent feature pairs ride one 4-byte lane;
     the stream transpose is column-rate-bound, so packing halves
     its cost).
  3. mm1: bf16, 8 K=32 steps (4 u32 blocks x even/odd lane via a
     stride-2 bf16 view) x 4 concurrent row groups with DIAGONAL
     tile_position (32P, 32P) into one [128, 1024] PSUM tile.
  4. Exact GELU (erf) on ScalarE over [128, 1024], b1 as
     per-partition bias, bf16 out.
  5. mm2 with CENTERED W2 (W2c[p,o] = W2[p,o] - mean_p W2[p,o]) so
     the matmul emits h2 - mean(h2) directly.  4 diagonal K=8
     matmuls.
  6. One DVE block transpose back to token-major; DVE squares and
     reduces -> per-token sum of squares.  (All non-DMA work stays
     off GpSimd: the Pool queue must remain a pure DMA-issue queue
     or the next SWDGE load head-of-line blocks behind
     compute-dependent ops.)
  7. Per round: ACT Sqrt (+eps, /8) -> DVE reciprocal -> DVE scale ->
     one 1 KB/partition store on the scalar HWDGE queue.

Targets the per-core HBM-read roofline: 32 MiB of x at the measured
per-engine descriptor rates ~= 95-100 us streaming, with compute
engines each budgeted under half the ~12.9 us/round DMA time.
"""

import os
import sys

import numpy as np

if not any(os.path.isdir(os.path.join(p, "concourse")) for p in sys.path if p):
    for _cand in ("/opt/trn_rl_repo", "/root/.axon_site/_ro/trn_rl_repo"):
        if os.path.isdir(os.path.join(_cand, "concourse")):
            sys.path.insert(0, _cand)
            break

N_CORES = 8
DIM, OUT = 256, 8
B, T = 128, 2048
TOK_TOTAL = B * T
TOK_CORE = TOK_TOTAL // N_CORES  # 32768
R_TOK = 4096                     # tokens per round
N_R = TOK_CORE // R_TOK          # 8 rounds
J = R_TOK // 128                 # 32 tokens per partition per round
JQ = J // 4                      # quarter-round slice
EPS = 1e-5

_BUILD_CACHE = {}


def build_kernel(use_b2c=False, use_gamma=False, use_beta=False):
    """Build the per-core Bass program. Returns the compiled Bacc object."""
    key = (use_b2c, use_gamma, use_beta)
    if key in _BUILD_CACHE:
        return _BUILD_CACHE[key]

    import concourse.bacc as bacc
    import concourse.mybir as mybir
    from concourse.tile import TileContext

    f32 = mybir.dt.float32
    bf16 = mybir.dt.bfloat16
    u32 = mybir.dt.uint32
    AF = mybir.ActivationFunctionType
    ALU = mybir.AluOpType

    nc = bacc.Bacc("TRN2")
    x_d = nc.dram_tensor("x", [TOK_CORE, DIM], f32, kind="ExternalInput")
    # bf16 consts: cols 0:256 w1t packed-lane blocks, 256:288 w2c diag
    wp16_d = nc.dram_tensor("wp16", [128, 288], bf16, kind="ExternalInput")
    # f32 consts: col 0 b1 diag; 8:16 b2-mean(b2); 16:24 gamma; 24:32 beta
    wpf_d = nc.dram_tensor("wpf", [128, 32], f32, kind="ExternalInput")
    y_d = nc.dram_tensor("y", [TOK_CORE, OUT], f32, kind="ExternalOutput")

    # token t = p*256 + r*32 + f
    x_v = x_d[:, :].rearrange("(p r f) d -> r p f d", p=128, r=N_R, f=J)
    # y view: partition p owns 256 consecutive tokens
    y_pv = y_d[:, :].rearrange("(p t) c -> p t c", p=128)

    with TileContext(nc) as tc:
        with (
            tc.tile_pool(name="consts", bufs=1) as consts,
            tc.tile_pool(name="xf0p", bufs=2) as xf0p,
            tc.tile_pool(name="xin", bufs=3) as xin,
            tc.tile_pool(name="xtp", bufs=2) as xtp,
            tc.tile_pool(name="h1p", bufs=2) as h1p,
            tc.tile_pool(name="sqp", bufs=2) as sqp,
            tc.tile_pool(name="accp", bufs=1) as accp,
            tc.tile_pool(name="yout", bufs=2) as yout,
            tc.tile_pool(name="pp", bufs=2, space="PSUM") as pp,
            tc.tile_pool(name="pp2", bufs=1, space="PSUM") as pp2,
        ):
            wp16 = consts.tile([128, 288], bf16)
            nc.sync.dma_start(out=wp16, in_=wp16_d[:, :])
            wpf = consts.tile([128, 32], f32)
            nc.sync.dma_start(out=wpf, in_=wpf_d[:, :])
            b1d = wpf[:, 0:1]
            eps_c = consts.tile([128, 1], f32)
            nc.vector.memset(eps_c, EPS)

            # persistent per-round state
            cent_all = accp.tile([128, N_R * J, 32], f32)   # 32 KB/part
            ssq_all = accp.tile([128, N_R * J], f32)

            def finalize(r):
                # rstd + scale + 1 KB/partition store for round r
                stdv = sqp.tile([128, J], f32, tag="stdv")
                nc.scalar.activation(
                    out=stdv,
                    in_=ssq_all[:, r * J : (r + 1) * J],
                    func=AF.Sqrt,
                    bias=eps_c[:, 0:1],
                    scale=1.0 / OUT,
                )
                rstd = sqp.tile([128, J], f32, tag="rstd")
                nc.vector.reciprocal(out=rstd, in_=stdv)
                y_t = yout.tile([128, J, 8], f32, tag="y_t")
                cent_b = cent_all[:, r * J : (r + 1) * J, 0:8]
                rs = rstd.rearrange("p (j c) -> p j c", c=1).broadcast_to(
                    [128, J, 8]
                )
                nc.vector.tensor_tensor(out=y_t, in0=cent_b, in1=rs, op=ALU.mult)
                if use_gamma:
                    gm = wpf[:, 16:24].rearrange(
                        "p (j c) -> p j c", j=1
                    ).broadcast_to([128, J, 8])
                    nc.vector.tensor_tensor(out=y_t, in0=y_t, in1=gm, op=ALU.mult)
                if use_beta:
                    bt = wpf[:, 24:32].rearrange(
                        "p (j c) -> p j c", j=1
                    ).broadcast_to([128, J, 8])
                    nc.vector.tensor_tensor(out=y_t, in0=y_t, in1=bt, op=ALU.add)
                nc.scalar.dma_start(out=y_pv[:, r * J : (r + 1) * J, :], in_=y_t)

            def process(r, x_sb, ps, ps2, h1, jl, jh):
                """Transpose + mm1 + GELU + mm2 + stats for token columns
                [jl, jh) of round r (jl/jh in units of J, 8-aligned)."""
                nj = jh - jl
                # ---- u32-packed 32x32 block transpose ----
                xt = xtp.tile([128, J, DIM // 2], u32, tag="xt")
                nc.vector.transpose(
                    out=xt[:, jl:jh, :],
                    in_=x_sb[:, jl:jh, :].bitcast(u32),
                )
                # xt16[32P+a, j, db2, c, e] = x[token (32P+c)*256 + J*r + j,
                #                              d = 64*db2 + 2a + e]
                xt16 = xt[:, :, :].bitcast(bf16).rearrange(
                    "p j (db2 c e) -> p j db2 c e", db2=4, c=32, e=2
                )

                # ---- mm1: 8 K=32 steps x 4 diagonal row groups ----
                for s in range(8):
                    db2, e = s // 2, s % 2
                    for P in range(4):
                        nc.tensor.matmul(
                            out=ps[32 * P : 32 * P + 32, 32 * jl : 32 * jh],
                            lhsT=wp16[32 * P : 32 * P + 32, 32 * s : 32 * s + 32],
                            rhs=xt16[32 * P : 32 * P + 32, jl:jh, db2, :, e],
                            start=(s == 0),
                            stop=(s == 7),
                            tile_position=(32 * P, 32 * P),
                            skip_group_check=True,
                        )

                # ---- exact GELU (erf) + b1, bf16 out ----
                nc.scalar.activation(
                    out=h1[:, 32 * jl : 32 * jh],
                    in_=ps[:, 32 * jl : 32 * jh],
                    func=AF.Gelu,
                    bias=b1d,
                    scale=1.0,
                )

                # ---- mm2 with centered W2: output IS h2 - mean(h2) ----
                for g in range(4):
                    nc.tensor.matmul(
                        out=ps2[32 * g : 32 * g + 32, 32 * jl : 32 * jh],
                        lhsT=wp16[32 * g : 32 * g + 8, 256:288],
                        rhs=h1[32 * g : 32 * g + 8, 32 * jl : 32 * jh],
                        start=True,
                        stop=True,
                        tile_position=(32 * g, 32 * g),
                        skip_group_check=True,
                    )

                # ---- back to token-major ----
                cent_r32 = cent_all[:, r * J + jl : r * J + jh, :]
                nc.vector.transpose(out=cent_r32, in_=ps2[:, 32 * jl : 32 * jh])
                cent_r = cent_all[:, r * J + jl : r * J + jh, 0:8]
                if use_b2c:
                    b2c = wpf[:, 8:16].rearrange(
                        "p (j c) -> p j c", j=1
                    ).broadcast_to([128, nj, 8])
                    nc.vector.tensor_tensor(
                        out=cent_r, in0=cent_r, in1=b2c, op=ALU.add
                    )

                # ---- sum of squares per token (DVE: keep Pool DMA-only) ----
                sq = sqp.tile([128, J, 8], f32, tag="sq")
                nc.vector.tensor_tensor(
                    out=sq[:, 0:nj, :], in0=cent_r, in1=cent_r, op=ALU.mult
                )
                nc.vector.reduce_sum(
                    out=ssq_all[:, r * J + jl : r * J + jh],
                    in_=sq[:, 0:nj, :],
                    axis=mybir.AxisListType.X,
                )

            for r in range(N_R):
                x_sb = xin.tile([128, J, DIM], bf16, tag="x_sb")
                ps = pp.tile([128, 32 * J], f32, tag="ps")
                ps2 = pp2.tile([128, 32 * J], f32, tag="ps2")
                h1 = h1p.tile([128, 32 * J], bf16, tag="h1")

                if r == 0:
                    # Sync HWDGE boots ~5 us before the Pool engine: pull
                    # round 0 as fp32 quarters on Sync + DVE casts so the
                    # stream starts at ~2.6 us and compute at ~7.
                    for q in range(4):
                        sl = slice(q * JQ, (q + 1) * JQ)
                        xf0 = xf0p.tile([128, JQ, DIM], f32, tag="xf0")
                        nc.sync.dma_start(out=xf0, in_=x_v[0][:, sl, :])
                        nc.vector.tensor_copy(out=x_sb[:, sl, :], in_=xf0)
                        process(r, x_sb, ps, ps2, h1, q * JQ, (q + 1) * JQ)
                elif r == N_R - 1:
                    # drain end: quarter loads so the tail pipeline gets
                    # its data piecewise
                    for q in range(4):
                        sl = slice(q * JQ, (q + 1) * JQ)
                        nc.gpsimd.dma_start(
                            out=x_sb[:, sl, :], in_=x_v[r][:, sl, :]
                        )
                        process(r, x_sb, ps, ps2, h1, q * JQ, (q + 1) * JQ)
                else:
                    # one 4 MB cast-DMA, 32 KB per-partition descriptors;
                    # compute in two 512-col units (matmul out must stay
                    # within a single 2 KB PSUM bank)
                    nc.gpsimd.dma_start(out=x_sb, in_=x_v[r])
                    process(r, x_sb, ps, ps2, h1, 0, J // 2)
                    process(r, x_sb, ps, ps2, h1, J // 2, J)

                finalize(r)

    nc.compile()
    _BUILD_CACHE[key] = nc
    return nc


def prep_inputs(x, W1, b1, W2, b2, gamma, beta):
    """Host-side prep: shard x, lay out the tiny weights for the kernel."""
    import ml_dtypes

    x = np.ascontiguousarray(np.asarray(x, dtype=np.float32)).reshape(TOK_TOTAL, DIM)
    W1 = np.asarray(W1, dtype=np.float32)
    b1 = np.asarray(b1, dtype=np.float32)
    W2 = np.asarray(W2, dtype=np.float32)
    b2 = np.asarray(b2, dtype=np.float32)
    gamma = np.asarray(gamma, dtype=np.float32)
    beta = np.asarray(beta, dtype=np.float32)

    # w1t[32P+a, 32s+b] = W1[b, 64*(s//2) + 2a + s%2] (b < 8), per P group
    s_idx = np.arange(8)
    a_idx = np.arange(32)
    d_idx = 64 * (s_idx[:, None] // 2) + 2 * a_idx[None, :] + s_idx[:, None] % 2
    w1g = np.zeros((32, 8, 32), np.float32)              # [a, s, bslot]
    w1g[:, :, :OUT] = W1[:, d_idx].transpose(2, 1, 0)    # [a, s, b]
    w1t = np.tile(w1g.reshape(32, DIM), (4, 1))

    # centered W2, diagonal placement: wp16[32g+o, 256+m] = W2c[m, o]
    W2c = W2 - W2.mean(axis=0, keepdims=True)            # [p, o] - mean_p
    w2d = np.zeros((128, 32), np.float32)
    for g in range(4):
        w2d[32 * g : 32 * g + OUT, :OUT] = W2c.T         # [o, m]

    wp16 = np.zeros((128, 288), np.float32)
    wp16[:, 0:DIM] = w1t
    wp16[:, DIM:288] = w2d
    wp16 = wp16.astype(ml_dtypes.bfloat16)

    wpf = np.zeros((128, 32), np.float32)
    for g in range(4):
        wpf[32 * g : 32 * g + OUT, 0] = b1               # diag bias for GELU
    wpf[:, 8:16] = (b2 - b2.mean())[None, :]
    wpf[:, 16:24] = gamma[None, :]
    wpf[:, 24:32] = beta[None, :]

    use_b2c = bool(np.any(b2 != 0.0))
    use_gamma = bool(np.any(gamma != 1.0))
    use_beta = bool(np.any(beta != 0.0))

    in_maps = []
    for k in range(N_CORES):
        m = {
            "x": np.ascontiguousarray(x[k * TOK_CORE : (k + 1) * TOK_CORE]),
            "wp16": wp16,
            "wpf": wpf,
        }
        in_maps.append(m)
    flags = dict(use_b2c=use_b2c, use_gamma=use_gamma, use_beta=use_beta)
    return in_maps, flags


def run(x, W1, b1, W2, b2, gamma, beta, trace=False, **kw):
    from concourse.bass_utils import run_bass_kernel_spmd

    in_maps, flags = prep_inputs(x, W1, b1, W2, b2, gamma, beta)
    nc = build_kernel(**flags)
    res = run_bass_kernel_spmd(
        nc, in_maps, core_ids=list(range(N_CORES)), trace=trace, **kw
    )
    y = np.concatenate([res.results[k]["y"] for k in range(N_CORES)], axis=0)
    return y.reshape(B, T, OUT).astype(np.float32), res


def kernel(x, W1, b1, W2, b2, gamma, beta):
    y, _ = run(x, W1, b1, W2, b2, gamma, beta)
    return y


# revision 18
# speedup vs baseline: 1.1562x; 1.1562x over previous
"""Trainium2 Bass kernel for nn_BottleneckFFN.

Computes y = LayerNorm(GELU(x @ W1.T + b1) @ W2.T + b2) * gamma + beta
for x of shape (128, 2048, 256), W1 (8, 256), W2 (8, 8), LN over the
trailing 8 channels.  Pure data parallel over 8 NeuronCores: the
128*2048 = 262144 token rows are split into 8 shards of 32768 tokens;
the tiny weights are replicated.

Token map inside a shard: t = p*256 + r*16 + f (partition p owns 256
consecutive tokens).  Per round the x load is one contiguous 16 KB run
per partition and the y stores batch 4 rounds into one contiguous 2 KB
run per partition.

Per-core dataflow (16 rounds of 2048 tokens):
  1. SWDGE (gpsimd) DMA casts 2 MB of fp32 x rows to bf16 on the fly
     while loading into SBUF, token-major [128 part, 16, 256].  The
     HBM read (the binding resource, ~358 GB/s/core) is unchanged;
     all on-chip traffic halves.
  2. One DVE 32x32 block transpose over the bf16 tile VIEWED AS
     uint32 (adjacent feature pairs ride one 4-byte lane).  The DVE
     stream transpose is column-rate-bound (~1.3 ns/element at any
     dtype), so packing pairs halves its cost vs transposing bf16
     elements directly.
  3. mm1: bf16 matmuls, 8 K=32 steps (4 u32 blocks x even/odd lane,
     the rhs is a stride-2 bf16 view) x 4 concurrent row groups with
     DIAGONAL tile_position (32P, 32P) accumulating into one
     [128, 512] PSUM bank.
  4. Exact GELU (erf) on ScalarE over [128, 512], b1 as per-partition
     bias, bf16 out.
  5. mm2 with CENTERED W2 (W2c[p,o] = W2[p,o] - mean_p W2[p,o]) so the
     matmul emits h2 - mean(h2) directly.  4 diagonal K=8 matmuls.
  6. One DVE block transpose back to token-major into a persistent
     cent buffer; gpsimd squares, DVE reduces -> per-token sum sq.
  7. Every 4 rounds: one ACT Sqrt (amortizes the Gelu<->Sqrt table
     swap), DVE reciprocal -> rstd; one gpsimd scale over the whole
     batch; one batched 2 KB/partition store on the scalar HWDGE
     queue.

Targets the per-core HBM-read roofline: 32 MiB of x at ~358 GB/s
~= 94 us; every other engine is budgeted under the ~5.9 us/round DMA.
"""

import os
import sys

import numpy as np

if not any(os.path.isdir(os.path.join(p, "concourse")) for p in sys.path if p):
    for _cand in ("/opt/trn_rl_repo", "/root/.axon_site/_ro/trn_rl_repo"):
        if os.path.isdir(os.path.join(_cand, "concourse")):
            sys.path.insert(0, _cand)
            break

N_CORES = 8
DIM, OUT = 256, 8
B, T = 128, 2048
TOK_TOTAL = B * T
TOK_CORE = TOK_TOTAL // N_CORES  # 32768
R_TOK = 2048                     # tokens per round
N_R = TOK_CORE // R_TOK          # 16 rounds
J = R_TOK // 128                 # 16 tokens per partition per round
FB = 4                           # rounds per finalize batch
EPS = 1e-5

_BUILD_CACHE = {}


def build_kernel(use_b2c=False, use_gamma=False, use_beta=False):
    """Build the per-core Bass program. Returns the compiled Bacc object."""
    key = (use_b2c, use_gamma, use_beta)
    if key in _BUILD_CACHE:
        return _BUILD_CACHE[key]

    import concourse.bacc as bacc
    import concourse.mybir as mybir
    from concourse.tile import TileContext

    f32 = mybir.dt.float32
    bf16 = mybir.dt.bfloat16
    u32 = mybir.dt.uint32
    AF = mybir.ActivationFunctionType
    ALU = mybir.AluOpType

    nc = bacc.Bacc("TRN2")
    x_d = nc.dram_tensor("x", [TOK_CORE, DIM], f32, kind="ExternalInput")
    # bf16 consts: cols 0:256 w1t packed-lane blocks, 256:288 w2c diag
    wp16_d = nc.dram_tensor("wp16", [128, 288], bf16, kind="ExternalInput")
    # f32 consts: col 0 b1 diag; 8:16 b2-mean(b2); 16:24 gamma; 24:32 beta
    wpf_d = nc.dram_tensor("wpf", [128, 32], f32, kind="ExternalInput")
    y_d = nc.dram_tensor("y", [TOK_CORE, OUT], f32, kind="ExternalOutput")

    # token t = p*256 + r*16 + f
    x_v = x_d[:, :].rearrange("(p r f) d -> r p f d", p=128, r=N_R, f=J)
    # y view: partition p owns 256 consecutive tokens; finalize stores
    # any contiguous [r_lo*16, r_hi*16) slice of them in one DMA.
    y_pv = y_d[:, :].rearrange("(p t) c -> p t c", p=128)

    with TileContext(nc) as tc:
        with (
            tc.tile_pool(name="consts", bufs=1) as consts,
            tc.tile_pool(name="xin", bufs=3) as xin,
            tc.tile_pool(name="xtp", bufs=2) as xtp,
            tc.tile_pool(name="h1p", bufs=2) as h1p,
            tc.tile_pool(name="sqp", bufs=2) as sqp,
            tc.tile_pool(name="accp", bufs=1) as accp,
            tc.tile_pool(name="yout", bufs=2) as yout,
            tc.tile_pool(name="pp", bufs=2, space="PSUM") as pp,
            tc.tile_pool(name="pp2", bufs=2, space="PSUM") as pp2,
        ):
            wp16 = consts.tile([128, 288], bf16)
            nc.sync.dma_start(out=wp16, in_=wp16_d[:, :])
            wpf = consts.tile([128, 32], f32)
            nc.sync.dma_start(out=wpf, in_=wpf_d[:, :])
            b1d = wpf[:, 0:1]
            eps_c = consts.tile([128, 1], f32)
            nc.vector.memset(eps_c, EPS)

            # persistent per-round state
            cent_all = accp.tile([128, N_R * J, 32], f32)   # 32 KB/part
            ssq_all = accp.tile([128, N_R * 16], f32)

            def finalize(r_lo, r_hi):
                # rstd + scale + batched store for rounds [r_lo, r_hi)
                nr = r_hi - r_lo
                stdv = sqp.tile([128, nr * 16], f32, tag="stdv")
                nc.scalar.activation(
                    out=stdv,
                    in_=ssq_all[:, r_lo * 16 : r_hi * 16],
                    func=AF.Sqrt,
                    bias=eps_c[:, 0:1],
                    scale=1.0 / OUT,
                )
                rstd = sqp.tile([128, nr * 16], f32, tag="rstd")
                nc.vector.reciprocal(out=rstd, in_=stdv)
                y_t = yout.tile([128, nr * J, 8], f32, tag="y_t")
                cent_b = cent_all[:, r_lo * J : r_hi * J, 0:8]
                rs = rstd.rearrange("p (j c) -> p j c", c=1).broadcast_to(
                    [128, nr * J, 8]
                )
                # DVE, not gpsimd: the Pool queue must stay a pure DMA-issue
                # queue or the next SWDGE load head-of-line blocks behind
                # compute-dependent tensor ops.
                nc.vector.tensor_tensor(out=y_t, in0=cent_b, in1=rs, op=ALU.mult)
                if use_gamma:
                    gm = wpf[:, 16:24].rearrange(
                        "p (j c) -> p j c", j=1
                    ).broadcast_to([128, nr * J, 8])
                    nc.vector.tensor_tensor(out=y_t, in0=y_t, in1=gm, op=ALU.mult)
                if use_beta:
                    bt = wpf[:, 24:32].rearrange(
                        "p (j c) -> p j c", j=1
                    ).broadcast_to([128, nr * J, 8])
                    nc.vector.tensor_tensor(out=y_t, in0=y_t, in1=bt, op=ALU.add)
                nc.scalar.dma_start(out=y_pv[:, r_lo * J : r_hi * J, :], in_=y_t)

            def process(r, x_sb, ps, ps2, h1, jl, jh):
                """Transpose + mm1 + GELU + mm2 + stats for token columns
                [jl, jh) of round r (jl/jh in units of J, 8-aligned)."""
                nj = jh - jl
                # ---- u32-packed 32x32 block transpose ----
                xt = xtp.tile([128, J, DIM // 2], u32, tag="xt")
                nc.vector.transpose(
                    out=xt[:, jl:jh, :],
                    in_=x_sb[:, jl:jh, :].bitcast(u32),
                )
                # xt16[32P+a, j, db2, c, e] = x[token (32P+c)*256 + 16r + j,
                #                              d = 64*db2 + 2a + e]
                xt16 = xt[:, :, :].bitcast(bf16).rearrange(
                    "p j (db2 c e) -> p j db2 c e", db2=4, c=32, e=2
                )

                # ---- mm1: 8 K=32 steps x 4 diagonal row groups ----
                for s in range(8):
                    db2, e = s // 2, s % 2
                    for P in range(4):
                        nc.tensor.matmul(
                            out=ps[32 * P : 32 * P + 32, 32 * jl : 32 * jh],
                            lhsT=wp16[32 * P : 32 * P + 32, 32 * s : 32 * s + 32],
                            rhs=xt16[32 * P : 32 * P + 32, jl:jh, db2, :, e],
                            start=(s == 0),
                            stop=(s == 7),
                            tile_position=(32 * P, 32 * P),
                            skip_group_check=True,
                        )

                # ---- exact GELU (erf) + b1, bf16 out ----
                nc.scalar.activation(
                    out=h1[:, 32 * jl : 32 * jh],
                    in_=ps[:, 32 * jl : 32 * jh],
                    func=AF.Gelu,
                    bias=b1d,
                    scale=1.0,
                )

                # ---- mm2 with centered W2: output IS h2 - mean(h2) ----
                for g in range(4):
                    nc.tensor.matmul(
                        out=ps2[32 * g : 32 * g + 32, 32 * jl : 32 * jh],
                        lhsT=wp16[32 * g : 32 * g + 8, 256:288],
                        rhs=h1[32 * g : 32 * g + 8, 32 * jl : 32 * jh],
                        start=True,
                        stop=True,
                        tile_position=(32 * g, 32 * g),
                        skip_group_check=True,
                    )

                # ---- back to token-major ----
                cent_r32 = cent_all[:, r * J + jl : r * J + jh, :]
                nc.vector.transpose(out=cent_r32, in_=ps2[:, 32 * jl : 32 * jh])
                cent_r = cent_all[:, r * J + jl : r * J + jh, 0:8]
                if use_b2c:
                    b2c = wpf[:, 8:16].rearrange(
                        "p (j c) -> p j c", j=1
                    ).broadcast_to([128, nj, 8])
                    nc.vector.tensor_tensor(
                        out=cent_r, in0=cent_r, in1=b2c, op=ALU.add
                    )

                # ---- sum of squares per token (DVE: keep Pool DMA-only) ----
                sq = sqp.tile([128, J, 8], f32, tag="sq")
                nc.vector.tensor_tensor(
                    out=sq[:, 0:nj, :], in0=cent_r, in1=cent_r, op=ALU.mult
                )
                nc.vector.reduce_sum(
                    out=ssq_all[:, r * 16 + jl : r * 16 + jh],
                    in_=sq[:, 0:nj, :],
                    axis=mybir.AxisListType.X,
                )

            # x loads in PAIRS of rounds: one 4 MB cast-DMA with 32 KB
            # per-partition descriptors (amortizes per-descriptor engine
            # overhead; tokens 32pr*1KB.. are contiguous per partition).
            # The last pair is split so the drain pipeline starts early.
            xv2 = x_d[:, :].rearrange(
                "(p pr f) d -> pr p f d", p=128, pr=N_R // 2, f=2 * J
            )
            for pr in range(N_R // 2):
                x2 = xin.tile([128, 2 * J, DIM], bf16, tag="x2")
                if pr == N_R // 2 - 1:
                    nc.gpsimd.dma_start(out=x2[:, 0:J, :], in_=xv2[pr][:, 0:J, :])
                    nc.gpsimd.dma_start(
                        out=x2[:, J : J + J // 2, :],
                        in_=xv2[pr][:, J : J + J // 2, :],
                    )
                    nc.gpsimd.dma_start(
                        out=x2[:, J + J // 2 :, :], in_=xv2[pr][:, J + J // 2 :, :]
                    )
                else:
                    nc.gpsimd.dma_start(out=x2, in_=xv2[pr])

                for half in range(2):
                    r = 2 * pr + half
                    x_sb = x2[:, half * J : (half + 1) * J, :]
                    ps = pp.tile([128, 512], f32, tag="ps")
                    ps2 = pp2.tile([128, 512], f32, tag="ps2")
                    h1 = h1p.tile([128, 512], bf16, tag="h1")
                    if r == N_R - 1:
                        # last round in two halves to shorten the drain tail
                        process(r, x_sb, ps, ps2, h1, 0, J // 2)
                        process(r, x_sb, ps, ps2, h1, J // 2, J)
                    else:
                        process(r, x_sb, ps, ps2, h1, 0, J)
                    if r % FB == FB - 1:
                        finalize(r - FB + 1, r + 1)

    nc.compile()
    _BUILD_CACHE[key] = nc
    return nc


def prep_inputs(x, W1, b1, W2, b2, gamma, beta):
    """Host-side prep: shard x, lay out the tiny weights for the kernel."""
    import ml_dtypes

    x = np.ascontiguousarray(np.asarray(x, dtype=np.float32)).reshape(TOK_TOTAL, DIM)
    W1 = np.asarray(W1, dtype=np.float32)
    b1 = np.asarray(b1, dtype=np.float32)
    W2 = np.asarray(W2, dtype=np.float32)
    b2 = np.asarray(b2, dtype=np.float32)
    gamma = np.asarray(gamma, dtype=np.float32)
    beta = np.asarray(beta, dtype=np.float32)

    # w1t[32P+a, 32s+b] = W1[b, 64*(s//2) + 2a + s%2] (b < 8), per P group
    s_idx = np.arange(8)
    a_idx = np.arange(32)
    d_idx = 64 * (s_idx[:, None] // 2) + 2 * a_idx[None, :] + s_idx[:, None] % 2
    w1g = np.zeros((32, 8, 32), np.float32)              # [a, s, bslot]
    w1g[:, :, :OUT] = W1[:, d_idx].transpose(2, 1, 0)    # [a, s, b]
    w1t = np.tile(w1g.reshape(32, DIM), (4, 1))

    # centered W2, diagonal placement: wp16[32g+o, 256+m] = W2c[m, o]
    W2c = W2 - W2.mean(axis=0, keepdims=True)            # [p, o] - mean_p
    w2d = np.zeros((128, 32), np.float32)
    for g in range(4):
        w2d[32 * g : 32 * g + OUT, :OUT] = W2c.T         # [o, m]

    wp16 = np.zeros((128, 288), np.float32)
    wp16[:, 0:DIM] = w1t
    wp16[:, DIM:288] = w2d
    wp16 = wp16.astype(ml_dtypes.bfloat16)

    wpf = np.zeros((128, 32), np.float32)
    for g in range(4):
        wpf[32 * g : 32 * g + OUT, 0] = b1               # diag bias for GELU
    wpf[:, 8:16] = (b2 - b2.mean())[None, :]
    wpf[:, 16:24] = gamma[None, :]
    wpf[:, 24:32] = beta[None, :]

    use_b2c = bool(np.any(b2 != 0.0))
    use_gamma = bool(np.any(gamma != 1.0))
    use_beta = bool(np.any(beta != 0.0))

    in_maps = []
    for k in range(N_CORES):
        m = {
            "x": np.ascontiguousarray(x[k * TOK_CORE : (k + 1) * TOK_CORE]),
            "wp16": wp16,
            "wpf": wpf,
        }
        in_maps.append(m)
    flags = dict(use_b2c=use_b2c, use_gamma=use_gamma, use_beta=use_beta)
    return in_maps, flags


def run(x, W1, b1, W2, b2, gamma, beta, trace=False, **kw):
    from concourse.bass_utils import run_bass_kernel_spmd

    in_maps, flags = prep_inputs(x, W1, b1, W2, b2, gamma, beta)
    nc = build_kernel(**flags)
    res = run_bass_kernel_spmd(
        nc, in_maps, core_ids=list(range(N_CORES)), trace=trace, **kw
    )
    y = np.concatenate([res.results[k]["y"] for k in range(N_CORES)], axis=0)
    return y.reshape(B, T, OUT).astype(np.float32), res


def kernel(x, W1, b1, W2, b2, gamma, beta):
    y, _ = run(x, W1, b1, W2, b2, gamma, beta)
    return y


# revision 21
# speedup vs baseline: 1.3846x; 1.1976x over previous
"""Trainium2 Bass kernel for nn_BottleneckFFN.

Computes y = LayerNorm(GELU(x @ W1.T + b1) @ W2.T + b2) * gamma + beta
for x of shape (128, 2048, 256), W1 (8, 256), W2 (8, 8), LN over the
trailing 8 channels.  Pure data parallel over 8 NeuronCores: the
128*2048 = 262144 token rows are split into 8 shards of 32768 tokens;
the tiny weights are replicated.

Token map inside a shard: t = p*256 + r*16 + f (partition p owns 256
consecutive tokens).  Per round the x load is one contiguous 16 KB run
per partition and the y stores batch 4 rounds into one contiguous 2 KB
run per partition.

Per-core dataflow (16 rounds of 2048 tokens):
  1. SWDGE (gpsimd) DMA casts 2 MB of fp32 x rows to bf16 on the fly
     while loading into SBUF, token-major [128 part, 16, 256].  The
     HBM read (the binding resource, ~358 GB/s/core) is unchanged;
     all on-chip traffic halves.
  2. One DVE 32x32 block transpose over the bf16 tile VIEWED AS
     uint32 (adjacent feature pairs ride one 4-byte lane).  The DVE
     stream transpose is column-rate-bound (~1.3 ns/element at any
     dtype), so packing pairs halves its cost vs transposing bf16
     elements directly.
  3. mm1: bf16 matmuls, 8 K=32 steps (4 u32 blocks x even/odd lane,
     the rhs is a stride-2 bf16 view) x 4 concurrent row groups with
     DIAGONAL tile_position (32P, 32P) accumulating into one
     [128, 512] PSUM bank.
  4. Exact GELU (erf) on ScalarE over [128, 512], b1 as per-partition
     bias, bf16 out.
  5. mm2 with CENTERED W2 (W2c[p,o] = W2[p,o] - mean_p W2[p,o]) so the
     matmul emits h2 - mean(h2) directly.  4 diagonal K=8 matmuls.
  6. One DVE block transpose back to token-major into a persistent
     cent buffer; gpsimd squares, DVE reduces -> per-token sum sq.
  7. Every 4 rounds: one ACT Sqrt (amortizes the Gelu<->Sqrt table
     swap), DVE reciprocal -> rstd; one gpsimd scale over the whole
     batch; one batched 2 KB/partition store on the scalar HWDGE
     queue.

Targets the per-core HBM-read roofline: 32 MiB of x at ~358 GB/s
~= 94 us; every other engine is budgeted under the ~5.9 us/round DMA.
"""

import os
import sys

import numpy as np

if not any(os.path.isdir(os.path.join(p, "concourse")) for p in sys.path if p):
    for _cand in ("/opt/trn_rl_repo", "/root/.axon_site/_ro/trn_rl_repo"):
        if os.path.isdir(os.path.join(_cand, "concourse")):
            sys.path.insert(0, _cand)
            break

N_CORES = 8
DIM, OUT = 256, 8
B, T = 128, 2048
TOK_TOTAL = B * T
TOK_CORE = TOK_TOTAL // N_CORES  # 32768
R_TOK = 2048                     # tokens per round
N_R = TOK_CORE // R_TOK          # 16 rounds
J = R_TOK // 128                 # 16 tokens per partition per round
FB = 4                           # rounds per finalize batch
EPS = 1e-5

_BUILD_CACHE = {}


def build_kernel(use_b2c=False, use_gamma=False, use_beta=False):
    """Build the per-core Bass program. Returns the compiled Bacc object."""
    key = (use_b2c, use_gamma, use_beta)
    if key in _BUILD_CACHE:
        return _BUILD_CACHE[key]

    import concourse.bacc as bacc
    import concourse.mybir as mybir
    from concourse.tile import TileContext

    f32 = mybir.dt.float32
    bf16 = mybir.dt.bfloat16
    u32 = mybir.dt.uint32
    AF = mybir.ActivationFunctionType
    ALU = mybir.AluOpType

    nc = bacc.Bacc("TRN2")
    x_d = nc.dram_tensor("x", [TOK_CORE, DIM], f32, kind="ExternalInput")
    # bf16 consts: cols 0:256 w1t packed-lane blocks, 256:288 w2c diag
    wp16_d = nc.dram_tensor("wp16", [128, 288], bf16, kind="ExternalInput")
    # f32 consts: col 0 b1 diag; 8:16 b2-mean(b2); 16:24 gamma; 24:32 beta
    wpf_d = nc.dram_tensor("wpf", [128, 32], f32, kind="ExternalInput")
    y_d = nc.dram_tensor("y", [TOK_CORE, OUT], f32, kind="ExternalOutput")

    # token t = p*256 + r*16 + f
    x_v = x_d[:, :].rearrange("(p r f) d -> r p f d", p=128, r=N_R, f=J)
    # y view: partition p owns 256 consecutive tokens; finalize stores
    # any contiguous [r_lo*16, r_hi*16) slice of them in one DMA.
    y_pv = y_d[:, :].rearrange("(p t) c -> p t c", p=128)

    with TileContext(nc) as tc:
        with (
            tc.tile_pool(name="consts", bufs=1) as consts,
            tc.tile_pool(name="xin", bufs=3) as xin,
            tc.tile_pool(name="xtp", bufs=2) as xtp,
            tc.tile_pool(name="h1p", bufs=2) as h1p,
            tc.tile_pool(name="sqp", bufs=2) as sqp,
            tc.tile_pool(name="accp", bufs=1) as accp,
            tc.tile_pool(name="yout", bufs=2) as yout,
            tc.tile_pool(name="pp", bufs=2, space="PSUM") as pp,
            tc.tile_pool(name="pp2", bufs=2, space="PSUM") as pp2,
        ):
            wp16 = consts.tile([128, 288], bf16)
            nc.sync.dma_start(out=wp16, in_=wp16_d[:, :])
            wpf = consts.tile([128, 32], f32)
            nc.sync.dma_start(out=wpf, in_=wpf_d[:, :])
            b1d = wpf[:, 0:1]
            eps_c = consts.tile([128, 1], f32)
            nc.vector.memset(eps_c, EPS)

            # persistent per-round state
            cent_all = accp.tile([128, N_R * J, 32], f32)   # 32 KB/part
            ssq_all = accp.tile([128, N_R * 16], f32)

            def finalize(r_lo, r_hi):
                # rstd + scale + batched store for rounds [r_lo, r_hi)
                nr = r_hi - r_lo
                stdv = sqp.tile([128, nr * 16], f32, tag="stdv")
                nc.scalar.activation(
                    out=stdv,
                    in_=ssq_all[:, r_lo * 16 : r_hi * 16],
                    func=AF.Sqrt,
                    bias=eps_c[:, 0:1],
                    scale=1.0 / OUT,
                )
                rstd = sqp.tile([128, nr * 16], f32, tag="rstd")
                nc.vector.reciprocal(out=rstd, in_=stdv)
                y_t = yout.tile([128, nr * J, 8], f32, tag="y_t")
                cent_b = cent_all[:, r_lo * J : r_hi * J, 0:8]
                rs = rstd.rearrange("p (j c) -> p j c", c=1).broadcast_to(
                    [128, nr * J, 8]
                )
                # DVE, not gpsimd: the Pool queue must stay a pure DMA-issue
                # queue or the next SWDGE load head-of-line blocks behind
                # compute-dependent tensor ops.
                nc.vector.tensor_tensor(out=y_t, in0=cent_b, in1=rs, op=ALU.mult)
                if use_gamma:
                    gm = wpf[:, 16:24].rearrange(
                        "p (j c) -> p j c", j=1
                    ).broadcast_to([128, nr * J, 8])
                    nc.vector.tensor_tensor(out=y_t, in0=y_t, in1=gm, op=ALU.mult)
                if use_beta:
                    bt = wpf[:, 24:32].rearrange(
                        "p (j c) -> p j c", j=1
                    ).broadcast_to([128, nr * J, 8])
                    nc.vector.tensor_tensor(out=y_t, in0=y_t, in1=bt, op=ALU.add)
                nc.scalar.dma_start(out=y_pv[:, r_lo * J : r_hi * J, :], in_=y_t)

            def front_mm(r, xt, ps, ps2, h1, jl, jh):
                """mm1 + GELU + mm2 for token columns [jl, jh) of round r,
                consuming an already-emitted packed transpose tile."""
                # xt16[32P+a, j, db2, c, e] = x[token (32P+c)*256 + 16r + j,
                #                              d = 64*db2 + 2a + e]
                xt16 = xt[:, :, :].bitcast(bf16).rearrange(
                    "p j (db2 c e) -> p j db2 c e", db2=4, c=32, e=2
                )

                # ---- mm1: 8 K=32 steps x 4 diagonal row groups ----
                for s in range(8):
                    db2, e = s // 2, s % 2
                    for P in range(4):
                        nc.tensor.matmul(
                            out=ps[32 * P : 32 * P + 32, 32 * jl : 32 * jh],
                            lhsT=wp16[32 * P : 32 * P + 32, 32 * s : 32 * s + 32],
                            rhs=xt16[32 * P : 32 * P + 32, jl:jh, db2, :, e],
                            start=(s == 0),
                            stop=(s == 7),
                            tile_position=(32 * P, 32 * P),
                            skip_group_check=True,
                        )

                # ---- exact GELU (erf) + b1, bf16 out ----
                nc.scalar.activation(
                    out=h1[:, 32 * jl : 32 * jh],
                    in_=ps[:, 32 * jl : 32 * jh],
                    func=AF.Gelu,
                    bias=b1d,
                    scale=1.0,
                )

                # ---- mm2 with centered W2: output IS h2 - mean(h2) ----
                for g in range(4):
                    nc.tensor.matmul(
                        out=ps2[32 * g : 32 * g + 32, 32 * jl : 32 * jh],
                        lhsT=wp16[32 * g : 32 * g + 8, 256:288],
                        rhs=h1[32 * g : 32 * g + 8, 32 * jl : 32 * jh],
                        start=True,
                        stop=True,
                        tile_position=(32 * g, 32 * g),
                        skip_group_check=True,
                    )

            def back(r, ps2, jl, jh):
                """PSUM transpose-back + per-token sum of squares."""
                nj = jh - jl
                # ---- back to token-major ----
                cent_r32 = cent_all[:, r * J + jl : r * J + jh, :]
                nc.vector.transpose(out=cent_r32, in_=ps2[:, 32 * jl : 32 * jh])
                cent_r = cent_all[:, r * J + jl : r * J + jh, 0:8]
                if use_b2c:
                    b2c = wpf[:, 8:16].rearrange(
                        "p (j c) -> p j c", j=1
                    ).broadcast_to([128, nj, 8])
                    nc.vector.tensor_tensor(
                        out=cent_r, in0=cent_r, in1=b2c, op=ALU.add
                    )

                # ---- sum of squares per token (DVE: keep Pool DMA-only) ----
                sq = sqp.tile([128, J, 8], f32, tag="sq")
                nc.vector.tensor_tensor(
                    out=sq[:, 0:nj, :], in0=cent_r, in1=cent_r, op=ALU.mult
                )
                nc.vector.reduce_sum(
                    out=ssq_all[:, r * 16 + jl : r * 16 + jh],
                    in_=sq[:, 0:nj, :],
                    axis=mybir.AxisListType.X,
                )

            # x loads in PAIRS of rounds: one 4 MB cast-DMA with 32 KB
            # per-partition descriptors (amortizes per-descriptor engine
            # overhead; tokens 32pr*1KB.. are contiguous per partition).
            # The last pair is split so the drain pipeline starts early.
            xv2 = x_d[:, :].rearrange(
                "(p pr f) d -> pr p f d", p=128, pr=N_R // 2, f=2 * J
            )
            x2_of = {}

            def ensure_pair(k):
                if k in x2_of or k >= N_R // 2:
                    return
                x2 = xin.tile([128, 2 * J, DIM], bf16, tag="x2")
                if k == N_R // 2 - 1:
                    nc.gpsimd.dma_start(out=x2[:, 0:J, :], in_=xv2[k][:, 0:J, :])
                    nc.gpsimd.dma_start(
                        out=x2[:, J : J + J // 2, :],
                        in_=xv2[k][:, J : J + J // 2, :],
                    )
                    nc.gpsimd.dma_start(
                        out=x2[:, J + J // 2 :, :], in_=xv2[k][:, J + J // 2 :, :]
                    )
                else:
                    nc.gpsimd.dma_start(out=x2, in_=xv2[k])
                x2_of[k] = x2

            # The x transpose for round r+1 is emitted BEFORE round r's
            # back half: engines are strict FIFOs, and without the hoist
            # the next transpose queues behind a PSUM transpose-back that
            # waits on the whole PE chain, degrading the round cadence
            # from the DMA pace to the chain latency.
            xt_parts = {}

            def emit_tps(r):
                if r >= N_R or r in xt_parts:
                    return
                ensure_pair(r // 2)
                x_sb = x2_of[r // 2][:, (r % 2) * J : (r % 2 + 1) * J, :]
                xt = xtp.tile([128, J, DIM // 2], u32, tag="xt")
                if r == N_R - 1:
                    # last round in halves to shorten the drain tail
                    parts = [(0, J // 2), (J // 2, J)]
                else:
                    parts = [(0, J)]
                for jl, jh in parts:
                    nc.vector.transpose(
                        out=xt[:, jl:jh, :],
                        in_=x_sb[:, jl:jh, :].bitcast(u32),
                    )
                xt_parts[r] = (xt, parts)

            for r in range(N_R):
                emit_tps(r)
                xt, parts = xt_parts[r]
                ps = pp.tile([128, 512], f32, tag="ps")
                ps2 = pp2.tile([128, 512], f32, tag="ps2")
                h1 = h1p.tile([128, 512], bf16, tag="h1")
                for jl, jh in parts:
                    front_mm(r, xt, ps, ps2, h1, jl, jh)
                emit_tps(r + 1)
                for jl, jh in parts:
                    back(r, ps2, jl, jh)
                if r % FB == FB - 1:
                    finalize(r - FB + 1, r + 1)

    nc.compile()
    _BUILD_CACHE[key] = nc
    return nc


def prep_inputs(x, W1, b1, W2, b2, gamma, beta):
    """Host-side prep: shard x, lay out the tiny weights for the kernel."""
    import ml_dtypes

    x = np.ascontiguousarray(np.asarray(x, dtype=np.float32)).reshape(TOK_TOTAL, DIM)
    W1 = np.asarray(W1, dtype=np.float32)
    b1 = np.asarray(b1, dtype=np.float32)
    W2 = np.asarray(W2, dtype=np.float32)
    b2 = np.asarray(b2, dtype=np.float32)
    gamma = np.asarray(gamma, dtype=np.float32)
    beta = np.asarray(beta, dtype=np.float32)

    # w1t[32P+a, 32s+b] = W1[b, 64*(s//2) + 2a + s%2] (b < 8), per P group
    s_idx = np.arange(8)
    a_idx = np.arange(32)
    d_idx = 64 * (s_idx[:, None] // 2) + 2 * a_idx[None, :] + s_idx[:, None] % 2
    w1g = np.zeros((32, 8, 32), np.float32)              # [a, s, bslot]
    w1g[:, :, :OUT] = W1[:, d_idx].transpose(2, 1, 0)    # [a, s, b]
    w1t = np.tile(w1g.reshape(32, DIM), (4, 1))

    # centered W2, diagonal placement: wp16[32g+o, 256+m] = W2c[m, o]
    W2c = W2 - W2.mean(axis=0, keepdims=True)            # [p, o] - mean_p
    w2d = np.zeros((128, 32), np.float32)
    for g in range(4):
        w2d[32 * g : 32 * g + OUT, :OUT] = W2c.T         # [o, m]

    wp16 = np.zeros((128, 288), np.float32)
    wp16[:, 0:DIM] = w1t
    wp16[:, DIM:288] = w2d
    wp16 = wp16.astype(ml_dtypes.bfloat16)

    wpf = np.zeros((128, 32), np.float32)
    for g in range(4):
        wpf[32 * g : 32 * g + OUT, 0] = b1               # diag bias for GELU
    wpf[:, 8:16] = (b2 - b2.mean())[None, :]
    wpf[:, 16:24] = gamma[None, :]
    wpf[:, 24:32] = beta[None, :]

    use_b2c = bool(np.any(b2 != 0.0))
    use_gamma = bool(np.any(gamma != 1.0))
    use_beta = bool(np.any(beta != 0.0))

    in_maps = []
    for k in range(N_CORES):
        m = {
            "x": np.ascontiguousarray(x[k * TOK_CORE : (k + 1) * TOK_CORE]),
            "wp16": wp16,
            "wpf": wpf,
        }
        in_maps.append(m)
    flags = dict(use_b2c=use_b2c, use_gamma=use_gamma, use_beta=use_beta)
    return in_maps, flags


def run(x, W1, b1, W2, b2, gamma, beta, trace=False, **kw):
    from concourse.bass_utils import run_bass_kernel_spmd

    in_maps, flags = prep_inputs(x, W1, b1, W2, b2, gamma, beta)
    nc = build_kernel(**flags)
    res = run_bass_kernel_spmd(
        nc, in_maps, core_ids=list(range(N_CORES)), trace=trace, **kw
    )
    y = np.concatenate([res.results[k]["y"] for k in range(N_CORES)], axis=0)
    return y.reshape(B, T, OUT).astype(np.float32), res


def kernel(x, W1, b1, W2, b2, gamma, beta):
    y, _ = run(x, W1, b1, W2, b2, gamma, beta)
    return y
